# revision 30
# baseline (speedup 1.0000x reference)
"""Trainium2 Bass kernel for dynamic low-pass filter decomposition.

Module: global-avg-pool -> 1x1 conv -> BN -> softmax over 3x3 taps gives a
per-(sample, group) 3x3 kernel; applied as a reflect-padded depthwise conv
over x; returns (low, x - low).

Sharding: data-parallel over batch n=8 across 8 NeuronCores (1 sample/core).

Per-core layout: partition p = c*2 + h (h = row-half of the image, c =
channel); each partition holds 98 rows x 192 cols (1 halo row above/below,
reflection resolved at DMA time by source-row choice) plus 1-elem front/back
pads so tap-shifted views stay in bounds.

All data stays bf16 end-to-end (rel-err budget 2e-2 >> bf16's 4e-3): input
DMA, the diagonal-matmul tap passes on the TensorEngine (PSUM f32), and
the low/high outputs (upcast to f32 on the host).  The center tap (shift 0)
is folded into the PSUM drain as a VectorE scalar_tensor_tensor
(low = w4*x + acc), so PE runs only 8 passes; the last small super-tiles
use a plain ScalarE PSUM cast instead so the final critical path avoids the
busy VectorE.  Reflect columns at w=0/191 come from a 9-matmul PE "edge
strip" spliced in with strided ScalarE copies; VectorE does high = x - low.
Global-sum partials are fused into the load phase (VectorE reduces on one
queue's tiles, ScalarE accum-activations on the other's).
"""
import sys
import os

sys.path.insert(0, "/opt/trn_rl_repo")

import numpy as np
from contextlib import ExitStack

import concourse.bass as bass
import concourse.tile as tile
from concourse import bacc, mybir
from concourse.bass_utils import run_bass_kernel_spmd

dt = mybir.dt
f32 = dt.float32
bf16 = dt.bfloat16

KS = 3
GROUP = 8
IC = 64
BN_EPS = 1e-5
N = 8
H = W = 192
RH = 96                 # rows per half-image
NB = 98 * W             # buffer elems per partition (98 rows of 192)
PAD = 1                 # front pad elems (also 1 at the back)
NST = 6                 # input-phase tiles
STW = 3072              # cols per input tile
CH = 512                # cols per PSUM accumulator chunk (ISA max)
ST_ROWS = [16, 16, 16, 16, 16, 10, 4, 2]   # compute super-tile heights
assert sum(ST_ROWS) == RH


def _chunks(stw):
    out = []
    a = 0
    while a < stw:
        c = min(CH, stw - a)
        out.append((a, c))
        a += c
    return out


def _build_program():
    """Trace the SPMD Bass program (same for every core)."""
    nc = bacc.Bacc("TRN2", target_bir_lowering=False, debug=False,
                   num_devices=N)

    x_d = nc.dram_tensor("x", [64, H, W], bf16, kind="ExternalInput")
    at_d = nc.dram_tensor("at128", [128, 72], f32, kind="ExternalInput")
    b_d = nc.dram_tensor("b72", [72, 1], f32, kind="ExternalInput")
    r9_d = nc.dram_tensor("r9", [72, 9], f32, kind="ExternalInput")
    g_d = nc.dram_tensor("g728", [72, 8], f32, kind="ExternalInput")
    h_d = nc.dram_tensor("h8128", [8, 128], f32, kind="ExternalInput")
    eye_d = nc.dram_tensor("eye", [128, 128], bf16, kind="ExternalInput")
    low_d = nc.dram_tensor("low", [64, H, W], bf16, kind="ExternalOutput")
    high_d = nc.dram_tensor("high", [64, H, W], bf16, kind="ExternalOutput")

    xt_dram = x_d.ap()

    def dram_flat(tensor, base, inner):
        """Flat (128, inner) AP over DRAM: partition p = c*2 + h covers
        x.flat[p*18432 : ...].  Flat APs keep descriptors large (6KB bf16
        rows) -> full 16-engine spray at ~26 GB/s/engine."""
        return bass.AP(tensor, base, [[RH * W, 128], [1, inner]])

    with tile.TileContext(nc) as tc, ExitStack() as ctx:
        cpool = ctx.enter_context(tc.tile_pool(name="consts", bufs=1))
        xpool = ctx.enter_context(tc.tile_pool(name="x", bufs=1))
        wpool = ctx.enter_context(tc.tile_pool(name="w", bufs=1))
        spool = ctx.enter_context(tc.tile_pool(name="stage", bufs=4))

        # ---- x ST loads FIRST (queue FIFO position = landing time) ----
        xb = xpool.tile([128, PAD + NB + 1], bf16)
        partials = wpool.tile([128, NST], f32)
        rscratch = wpool.tile([128, 3840], f32)
        IN_W = [3840, 3840, 1536]
        IN_TILES = []   # (flat offset, width, queue id)
        off = 0
        for q in range(2):
            for w_ in IN_W:
                IN_TILES.append((off, w_, q))
                off += w_
        assert off == RH * W
        for off, w_, q in IN_TILES:
            a = PAD + W + off
            eng = (nc.sync, nc.scalar)[q]
            eng.dma_start(xb[:, a:a + w_],
                          dram_flat(xt_dram.tensor, off, w_))
        # halo row 0 <- image rows {1 (reflect), 95}[h]
        nc.sync.dma_start(xb[:, PAD:PAD + W],
                          bass.AP(xt_dram.tensor, W,
                                  [[H * W, 64], [94 * W, 2], [1, W]]))
        # halo row 97 <- image rows {96, 190 (reflect)}[h]
        nc.sync.dma_start(xb[:, PAD + 97 * W:PAD + 98 * W],
                          bass.AP(xt_dram.tensor, 96 * W,
                                  [[H * W, 64], [94 * W, 2], [1, W]]))

        # ---- constant loads (gpsimd SWDGE; Pool engine is otherwise idle,
        # keeps sync/scalar HWDGE queues clear for the x STs) ----
        at_s = cpool.tile([128, 72], f32)
        b_s = cpool.tile([72, 1], f32)
        r9_s = cpool.tile([72, 9], f32)
        g_s = cpool.tile([72, 8], f32)
        h_s = cpool.tile([8, 128], f32)
        eye_s = cpool.tile([128, 128], bf16)
        for t, d in ((at_s, at_d), (b_s, b_d), (r9_s, r9_d), (g_s, g_d),
                     (h_s, h_d), (eye_s, eye_d)):
            nc.gpsimd.dma_start(t[:], d.ap())

        # ---- per-tile global-sum partials (vector reduce / scalar accum) ----
        for s, (off, w_, q) in enumerate(IN_TILES):
            a = PAD + W + off
            if q == 0:
                nc.vector.tensor_reduce(partials[:, s:s + 1],
                                        xb[:, a:a + w_],
                                        axis=mybir.AxisListType.X,
                                        op=mybir.AluOpType.add)
            else:
                nc.scalar.activation(rscratch[:, 0:w_], xb[:, a:a + w_],
                                     mybir.ActivationFunctionType.Copy,
                                     accum_out=partials[:, s:s + 1])

        # ---- weight generation ----
        sum128 = wpool.tile([128, 1], f32)
        nc.vector.tensor_reduce(sum128[:], partials[:],
                                axis=mybir.AxisListType.X,
                                op=mybir.AluOpType.add)
        with tc.tile_pool(name="wpsum", bufs=1,
                          space=bass.MemorySpace.PSUM) as wpsum:
            lf_p = wpsum.tile([72, 1], f32, tag="lf")
            nc.tensor.matmul(lf_p[:], at_s[:], sum128[:])
            e72 = wpool.tile([72, 1], f32)
            nc.scalar.activation(e72[:], lf_p[:],
                                 mybir.ActivationFunctionType.Exp,
                                 bias=b_s[:, 0:1], scale=1.0)
            rhsw = wpool.tile([72, 9], f32)
            nc.vector.tensor_scalar_mul(rhsw[:], r9_s[:], e72[:, 0:1])
            w89_p = wpsum.tile([8, 9], f32, tag="w89")
            nc.tensor.matmul(w89_p[:], g_s[:], rhsw[:])
            s8 = wpool.tile([8, 1], f32)
            nc.vector.tensor_reduce(s8[:], w89_p[:],
                                    axis=mybir.AxisListType.X,
                                    op=mybir.AluOpType.add)
            r8 = wpool.tile([8, 1], f32)
            nc.vector.reciprocal(r8[:], s8[:])
            w89s = wpool.tile([8, 9], f32)
            nc.vector.tensor_scalar_mul(w89s[:], w89_p[:], r8[:, 0:1])
            wbig_p = wpsum.tile([128, 9], f32)
            nc.tensor.matmul(wbig_p[:], h_s[:], w89s[:])
            w128 = wpool.tile([128, 9], f32)
            nc.scalar.copy(w128[:], wbig_p[:])

            # diagonal weight matrices (bf16), one tile per tap; split
            # across DVE (PSUM-sourced scalar) and ScalarE (w128-sourced)
            # so the 9 builds take ~5 serial slots instead of 9
            diag = [wpool.tile([128, 128], bf16, name=f"diag{k}")
                    for k in range(9)]
            for k in range(9):
                if k % 2 == 0:
                    nc.vector.tensor_scalar_mul(diag[k][:], eye_s[:],
                                                wbig_p[:, k:k + 1])
                else:
                    nc.scalar.activation(diag[k][:], eye_s[:],
                                         mybir.ActivationFunctionType.Copy,
                                         scale=w128[:, k:k + 1])

        with tc.tile_pool(name="psum", bufs=7,
                          space=bass.MemorySpace.PSUM) as psum, \
             tc.tile_pool(name="strip", bufs=1,
                          space=bass.MemorySpace.PSUM) as strippool:
            # ---- edge strip: correct w=0/191 values for all 96 rows ----
            # per tap: w=0 uses source col (1,0,1)[dj]; w=191 uses
            # (190,191,190)[dj]; strip layout [96 rows, 2].  Issued after
            # ST0's matmuls so PE start only gates on diag0, not all nine.
            strip = strippool.tile([128, 2 * RH], f32)

            strip_v = strip[:].rearrange("p (r e) -> p r e", e=2)
            for k in range(9):
                di, dj = k // 3, k % 3
                wl = (1, 0, 1)[dj]
                wr = (190, 191, 190)[dj]
                vb = PAD + di * W + wl
                view = xb[:, vb:vb + RH * W].rearrange(
                    "p (r w) -> p r w", w=W)[:, :, 0:wr - wl + 1:wr - wl]
                nc.tensor.matmul(strip_v, diag[k][:], view,
                                 start=(k == 0), stop=(k == 8))

            # ---- main loop: all taps on PE ----
            r0 = 0
            for si, rows in enumerate(ST_ROWS):
                stw = rows * W
                base = PAD + W + r0 * W
                chs = _chunks(stw)
                low_st = spool.tile([128, stw], bf16, tag="low",
                                    padded_shape=[128, STW])
                high_st = spool.tile([128, stw], bf16, tag="high",
                                     padded_shape=[128, STW])
                acc = [psum.tile([128, cw], f32, tag="acc",
                                 name=f"acc{si}_{i}", padded_shape=[128, CH])
                       for i, (a, cw) in enumerate(chs)]
                taps = list(range(9) if si % 2 == 0 else range(8, -1, -1))
                fuse_center = si < len(ST_ROWS) - 3
                if fuse_center:
                    taps.remove(4)   # center tap folds into the drain STT
                for k in taps:
                    di, dj = k // 3, k % 3
                    shift = (di - 1) * W + (dj - 1)
                    for ci, (a, cw) in enumerate(chs):
                        off = base + a + shift
                        nc.tensor.matmul(acc[ci][:], diag[k][:],
                                         xb[:, off:off + cw],
                                         start=(k == taps[0]),
                                         stop=(k == taps[-1]))
                # PSUM drain: fused center-tap STT on DVE for the bulk,
                # plain ScalarE cast for the tail STs (keeps the final
                # critical path off the busy VectorE)
                for ci, (a, cw) in enumerate(chs):
                    if fuse_center:
                        nc.vector.scalar_tensor_tensor(
                            low_st[:, a:a + cw],
                            xb[:, base + a:base + a + cw],
                            w128[:, 4:5], acc[ci][:],
                            op0=mybir.AluOpType.mult,
                            op1=mybir.AluOpType.add)
                    else:
                        nc.scalar.copy(low_st[:, a:a + cw], acc[ci][:])

                def finish(si=si, r0=r0, rows=rows, stw=stw, base=base,
                           low_st=low_st, high_st=high_st):
                    # splice in the correct reflect columns (strided copy)
                    edge_dst = low_st[:, 0:stw].rearrange(
                        "p (r w) -> p r w", w=W)[:, :, 0:W:W - 1]
                    edge_src = strip[:, 2 * r0:2 * (r0 + rows)].rearrange(
                        "p (r e) -> p r e", e=2)
                    nc.scalar.copy(edge_dst, edge_src)
                    nc.vector.tensor_tensor(high_st[:],
                                            xb[:, base:base + stw],
                                            low_st[:],
                                            op=mybir.AluOpType.subtract)
                    nc.sync.dma_start(
                        dram_flat(low_d.ap().tensor, r0 * W, stw), low_st[:])
                    nc.scalar.dma_start(
                        dram_flat(high_d.ap().tensor, r0 * W, stw),
                        high_st[:])

                finish()
                r0 += rows

    nc.compile()
    return nc


_nc_cache = None


def _get_program():
    global _nc_cache
    if _nc_cache is None:
        _nc_cache = _build_program()
    return _nc_cache


def _host_consts(conv_w, bn_gamma, bn_beta, bn_mean, bn_var):
    import ml_dtypes
    s_a = bn_gamma / np.sqrt(bn_var + BN_EPS)
    b72 = (bn_beta - bn_mean * s_a).astype(np.float32).reshape(72, 1)
    A = (conv_w * s_a[:, None]) / np.float32(H * W)
    p = np.arange(128)
    at128 = np.ascontiguousarray(A.T[p // 2]).astype(np.float32)  # (128, 72)
    oc = np.arange(72)
    r9 = (oc[:, None] % 9 == np.arange(9)[None, :]).astype(np.float32)
    g728 = (oc[:, None] // 9 == np.arange(8)[None, :]).astype(np.float32)
    h8128 = (np.arange(8)[:, None] == (p[None, :] // 16)).astype(np.float32)
    eye = np.eye(128, dtype=ml_dtypes.bfloat16)
    return dict(at128=at128, b72=b72, r9=r9, g728=g728, h8128=h8128, eye=eye)


def _prep_in_maps(x, conv_w, bn_gamma, bn_beta, bn_mean, bn_var):
    import ml_dtypes
    xb = np.ascontiguousarray(np.asarray(x, np.float32)).astype(
        ml_dtypes.bfloat16)
    consts = _host_consts(np.asarray(conv_w, np.float32),
                          np.asarray(bn_gamma, np.float32),
                          np.asarray(bn_beta, np.float32),
                          np.asarray(bn_mean, np.float32),
                          np.asarray(bn_var, np.float32))
    return [dict(x=xb[i], **consts) for i in range(N)]


def kernel(x, conv_w, bn_gamma, bn_beta, bn_mean, bn_var):
    in_maps = _prep_in_maps(x, conv_w, bn_gamma, bn_beta, bn_mean, bn_var)
    nc = _get_program()
    res = run_bass_kernel_spmd(nc, in_maps, list(range(N))).results
    low = np.stack([np.asarray(res[i]["low"]) for i in range(N)])
    high = np.stack([np.asarray(res[i]["high"]) for i in range(N)])
    return low.astype(np.float32), high.astype(np.float32)


if __name__ == "__main__":
    rng = np.random.default_rng(0)
    demo = dict(
        x=rng.standard_normal((N, IC, H, W), dtype=np.float32),
        conv_w=rng.standard_normal((72, 64)).astype(np.float32),
        bn_gamma=np.ones(72, np.float32),
        bn_beta=np.zeros(72, np.float32),
        bn_mean=rng.standard_normal(72).astype(np.float32) * 0.1,
        bn_var=rng.uniform(0.5, 1.5, 72).astype(np.float32),
    )
    low, high = kernel(**demo)
    print("ok", low.shape, high.shape)
